# revision 1
# baseline (speedup 1.0000x reference)
# Trainium2 Bass kernel for nn_CapLayer (CapsNet grouped 1x1 conv + dynamic routing).
#
# Key algebraic restructuring: the huge intermediate pred[b, i=(g,s), (j,d)]
# (188MB for the full batch) is NEVER materialized. Routing is computed in a
# factored form:
#   pred[b,(g,s),(j,d)] = sum_c Wa[g,j,d,c] * xga[b,g,c,s]     (c augmented with
#                                                               a ones channel to
#                                                               absorb the bias)
#   t[b,j,g,c]  = sum_s c[b,j,(g,s)] * xga[b,g,c,s]
#   s[b,j,d]    = sum_{g,c} t[b,j,g,c] * Wa[g,j,d,c]
#   u[b,j,g,c]  = sum_d v[b,j,d] * Wa[g,j,d,c]
#   db[b,j,g,s] = sum_c u[b,j,g,c] * xga[b,g,c,s]
# Iteration 1 collapses (softmax of zeros is uniform): t1 = xsum / J.
#
# Sharding: pure data parallel, 32 samples per core across 8 cores.
# On-chip layout: partition p = (b4, g) with 4 samples x 32 groups = 128
# partitions; 8 chunks cover the 32 local samples. The g-contraction for
# s[b,(j,d)] is done on the TensorEngine with a block-diagonal ones matrix,
# which also replicates s across the g-partitions for free (so v and u stay
# in the same partition layout).
#
# Engine split: broadcast-products run in bf16 (DVE 2x mode / GPSIMD),
# segmented reductions and small elementwise stay on DVE in fp32 accuracy,
# exp/sqrt/copies ride the Scalar engine, the g-sum is a TensorE matmul.

import sys

import numpy as np

# concourse (Bass/Tile) ships with the container; make sure it's importable
# when the grader runs kernel.py from a bare directory.
for _p in ("/opt/trn_rl_repo", "/root/.axon_site/_ro/trn_rl_repo"):
    if _p not in sys.path:
        sys.path.insert(0, _p)

NS, J, D, C_IN, H, WID, RN = 32, 10, 16, 8, 6, 6, 3
S = H * WID            # 36 spatial positions
CA = C_IN + 1          # 9 channels including the ones channel
CP = 10                # padded channel stride (4B alignment for bf16 rows)
NCORES = 8
BLOC = 32              # samples per core
B4 = 4                 # samples per chunk
NCH = BLOC // B4       # 8 chunks

_CACHE = {}


def _build_program(split_waits=True, dve_chunks=8, dma_eng="sync"):
    from contextlib import ExitStack

    import concourse.bass as bass
    import concourse.tile as tile
    from concourse import mybir

    f32 = mybir.dt.float32
    bf16 = mybir.dt.float16
    Alu = mybir.AluOpType
    Act = mybir.ActivationFunctionType
    AxX = mybir.AxisListType.X

    nc = bass.Bass("TRN2", target_bir_lowering=True, debug=False,
                   num_devices=NCORES)

    xcs_d = nc.dram_tensor("xcs", [NCH, 128, CA * S], bf16,
                           kind="ExternalInput").ap()      # free = (c, s)
    xsc_d = nc.dram_tensor("xsc", [NCH, 128, S * CP], bf16,
                           kind="ExternalInput").ap()      # free = (s, c10)
    wc_d = nc.dram_tensor("wc", [128, J * D * CP], bf16,
                          kind="ExternalInput").ap()       # free = (j, d, c10)
    wu_d = nc.dram_tensor("wu", [128, J * CA * D], bf16,
                          kind="ExternalInput").ap()       # free = (j, c, d)
    ones_d = nc.dram_tensor("onesb", [128, 128], bf16,
                            kind="ExternalInput").ap()     # blockdiag over b4
    v_d = nc.dram_tensor("v", [BLOC, J * D], f32,
                         kind="ExternalOutput").ap()

    dmae = {"gpsimd": nc.gpsimd, "sync": nc.sync}[dma_eng]
    with tile.TileContext(nc) as tc, ExitStack() as ctx:
        consts = ctx.enter_context(tc.tile_pool(name="consts", bufs=1))
        xpool = ctx.enter_context(tc.tile_pool(name="xpool", bufs=1))
        lpool = ctx.enter_context(tc.tile_pool(name="lpool", bufs=1))
        spool = ctx.enter_context(tc.tile_pool(name="scratch", bufs=2))
        small = ctx.enter_context(tc.tile_pool(name="small", bufs=3))
        vpool = ctx.enter_context(tc.tile_pool(name="vpool", bufs=2))
        psum = ctx.enter_context(tc.tile_pool(name="psum", bufs=2,
                                              space="PSUM"))

        wc_t = consts.tile([128, J * D * CP], bf16, tag="wc")
        dmae.dma_start(wc_t[:, :], wc_d[:, :])
        wu_t = consts.tile([128, J * CA * D], bf16, tag="wu")
        dmae.dma_start(wu_t[:, :], wu_d[:, :])
        ones_t = consts.tile([128, 128], bf16, tag="onesb")
        dmae.dma_start(ones_t[:, :], ones_d[:, :])

        # Persistent per-chunk tiles.
        Xcs = []   # xga [p, (c, s)] bf16
        Xsc = []   # xga [p, (s, c)] bf16
        L = []     # routing logits b, layout [p, (j, s)] fp32
        for ch in range(NCH):
            xt = xpool.tile([128, CA * S], bf16, tag=f"Xcs{ch}",
                            name=f"Xcs{ch}")
            dmae.dma_start(xt[:, :], xcs_d[ch, :, :])
            Xcs.append(xt)
            xt2 = xpool.tile([128, S * CP], bf16, tag=f"Xsc{ch}",
                             name=f"Xsc{ch}")
            dmae.dma_start(xt2[:, :], xsc_d[ch, :, :])
            Xsc.append(xt2)
            L.append(lpool.tile([128, J * S], f32, tag=f"L{ch}",
                                name=f"L{ch}"))

        def prod_engine(ch):
            # Split the broadcast-product work between DVE and GPSIMD by
            # chunk so both engines stay busy.
            return nc.vector if (ch % 8) < dve_chunks else nc.gpsimd

        def c_step(ch, t_in0_bcast):
            """t x Wa summed over (g, c) -> replicated s [p, (j,d)].

            t_in0_bcast: AP broadcast to [p, J, D, CA] (bf16).
            Returns an SBUF tile [128, J*D] fp32 with s replicated over g
            within each b4 partition block.
            """
            eng = prod_engine(ch)
            pc = spool.tile([128, J * D * CP], bf16, tag="prodC")
            pc4 = (pc[:, :].rearrange("p (j d c) -> p j d c", j=J, d=D)
                   [:, :, :, 0:CA])
            wc4 = (wc_t[:, :].rearrange("p (j d c) -> p j d c", j=J, d=D)
                   [:, :, :, 0:CA])
            eng.tensor_tensor(pc4, t_in0_bcast, wc4, Alu.mult)
            # PE contracts g (partitions, via blockdiag ones) AND c (PSUM
            # accumulation over the 9 channel slices) in one group -- no
            # DVE reduction needed at all.
            pcz = pc[:, :].rearrange("p (a c) -> p a c", c=CP)
            ps = psum.tile([128, J * D], f32, tag="psum_s")
            for c in range(CA):
                nc.tensor.matmul(ps[:, :], ones_t[:, :], pcz[:, :, c],
                                 start=(c == 0), stop=(c == CA - 1))
            s_sb = small.tile([128, J * D], f32, tag="s_sb")
            nc.scalar.copy(s_sb[:, :], ps[:, :])
            return ps, s_sb

        def squash(ch, s_ps, s_sb, want_bf16):
            """v = s * |s| / (1 + |s|^2), norm over d."""
            s2 = small.tile([128, J * D], f32, tag="s2")
            nc.scalar.activation(s2[:, :], s_ps[:, :], Act.Square)
            n2 = small.tile([128, J], f32, tag="n2")
            nc.vector.tensor_reduce(
                n2[:, :], s2[:, :].rearrange("p (j d) -> p j d", j=J), AxX,
                Alu.add)
            n2p1 = small.tile([128, J], f32, tag="n2p1")
            nc.scalar.add(n2p1[:, :], n2[:, :], 1.0)
            r = small.tile([128, J], f32, tag="rcp")
            nc.vector.reciprocal(r[:, :], n2p1[:, :])
            nr = small.tile([128, J], f32, tag="nrm")
            nc.scalar.activation(nr[:, :], n2[:, :], Act.Sqrt)
            f = small.tile([128, J], f32, tag="fac")
            nc.vector.tensor_tensor(f[:, :], nr[:, :], r[:, :], Alu.mult)
            fb = f[:, :].unsqueeze(2).broadcast_to([128, J, D])
            if want_bf16:
                vt = vpool.tile([128, J * D], bf16, tag="vtb")
            else:
                vt = vpool.tile([128, J * D], f32, tag="vtf")
            nc.vector.tensor_tensor(
                vt[:, :].rearrange("p (j d) -> p j d", j=J),
                s_sb[:, :].rearrange("p (j d) -> p j d", j=J), fb, Alu.mult)
            return vt

        def u_step(ch, vt):
            """u[p,(j,c)] = sum_d v[p,(j,d)] * Wa[p,(j,c,d)]. Out bf16
            padded to stride CP."""
            eng = prod_engine(ch)
            pu = spool.tile([128, J * CA * D], bf16, tag="produ")
            pu4 = pu[:, :].rearrange("p (j c d) -> p j c d", j=J, c=CA)
            wu4 = wu_t[:, :].rearrange("p (j c d) -> p j c d", j=J, c=CA)
            vb = (vt[:, :].rearrange("p (j d) -> p j d", j=J)
                  .unsqueeze(2).broadcast_to([128, J, CA, D]))
            eng.tensor_tensor(pu4, vb, wu4, Alu.mult)
            puz = pu[:, :].rearrange("p (a d) -> p a d", d=D)
            uA = spool.tile([128, 90 * 8], bf16, tag="treeUA")
            uA3 = uA[:, :].rearrange("p (a c) -> p a c", c=8)
            nc.vector.tensor_tensor(uA3, puz[:, :, 0:8], puz[:, :, 8:16],
                                    Alu.add)
            uB = spool.tile([128, 90 * 4], bf16, tag="treeUB")
            uB3 = uB[:, :].rearrange("p (a c) -> p a c", c=4)
            nc.vector.tensor_tensor(uB3, uA3[:, :, 0:4], uA3[:, :, 4:8],
                                    Alu.add)
            uC = spool.tile([128, 90 * 2], bf16, tag="treeUC")
            uC3 = uC[:, :].rearrange("p (a c) -> p a c", c=2)
            nc.vector.tensor_tensor(uC3, uB3[:, :, 0:2], uB3[:, :, 2:4],
                                    Alu.add)
            u = small.tile([128, J * CP], bf16, tag="u")
            u3 = u[:, :].rearrange("p (j c) -> p j c", j=J)[:, :, 0:CA]
            nc.vector.tensor_tensor(u3, uC3[:, :, 0], uC3[:, :, 1],
                                    Alu.add)
            return u

        def e_heavy(ch, u, out_js):
            """db[p,(j,s)] = sum_c u[p,(j,c)] * x[p,(s,c)] -> out_js fp32."""
            eng = prod_engine(ch)
            pe = spool.tile([128, J * S * CP], bf16, tag="prodE")
            pe4 = (pe[:, :].rearrange("p (j s c) -> p j s c", j=J, s=S)
                   [:, :, :, 0:CA])
            ub = (u[:, :].rearrange("p (j c) -> p j c", j=J)[:, :, 0:CA]
                  .unsqueeze(2).broadcast_to([128, J, S, CA]))
            xb = (Xsc[ch][:, :].rearrange("p (s c) -> p s c", s=S)
                  [:, :, 0:CA].unsqueeze(1)
                  .broadcast_to([128, J, S, CA]))
            eng.tensor_tensor(pe4, ub, xb, Alu.mult)
            # pe layout (j, s, c10): (j,s) merges; tree-sum over c
            pez = pe[:, :].rearrange("p (a c) -> p a c", c=CP)
            eA = spool.tile([128, 360 * 4], bf16, tag="treeEA")
            eA3 = eA[:, :].rearrange("p (a c) -> p a c", c=4)
            nc.vector.tensor_tensor(eA3, pez[:, :, 0:4], pez[:, :, 4:8],
                                    Alu.add)
            eB = spool.tile([128, 360 * 2], bf16, tag="treeEB")
            eB3 = eB[:, :].rearrange("p (a c) -> p a c", c=2)
            nc.vector.tensor_tensor(eB3, eA3[:, :, 0:2], eA3[:, :, 2:4],
                                    Alu.add)
            nc.vector.tensor_tensor(out_js, eB3[:, :, 0], eB3[:, :, 1],
                                    Alu.add)
            nc.vector.scalar_tensor_tensor(out_js, pez[:, :, 8], 1.0,
                                           out_js, Alu.mult, Alu.add)

        def softmax(ch):
            """c[p,(j,s)] = softmax_j(L). Returns bf16 C tile."""
            et = spool.tile([128, J * S], f32, tag="expt")
            nc.scalar.activation(et[:, :], L[ch][:, :], Act.Exp)
            z = small.tile([128, S], f32, tag="z")
            # reduce over j: view [p, s(outer, stride 1), j(inner, stride S)]
            ejs = (et[:, :].rearrange("p (j s) -> p j s", j=J)
                   .transpose([0, 2, 1]))
            nc.vector.tensor_reduce(z[:, :], ejs, AxX, Alu.add)
            zr = small.tile([128, S], f32, tag="zr")
            nc.vector.reciprocal(zr[:, :], z[:, :])
            ct = spool.tile([128, J * S], bf16, tag="ct")
            zb = zr[:, :].unsqueeze(1).broadcast_to([128, J, S])
            nc.vector.tensor_tensor(
                ct[:, :].rearrange("p (j s) -> p j s", j=J),
                et[:, :].rearrange("p (j s) -> p j s", j=J), zb, Alu.mult)
            return ct

        def b_heavy(ch, ct):
            """t[p,(j,c)] = sum_s c[p,(j,s)] * x[p,(c,s)]. Out bf16 padded
            to stride CP."""
            eng = prod_engine(ch)
            pb = spool.tile([128, J * CA * S], bf16, tag="prodB")
            pb4 = pb[:, :].rearrange("p (j c s) -> p j c s", j=J, c=CA)
            cb = (ct[:, :].rearrange("p (j s) -> p j s", j=J)
                  .unsqueeze(2).broadcast_to([128, J, CA, S]))
            xb = (Xcs[ch][:, :].rearrange("p (c s) -> p c s", c=CA)
                  .unsqueeze(1).broadcast_to([128, J, CA, S]))
            eng.tensor_tensor(pb4, cb, xb, Alu.mult)
            pbz = pb[:, :].rearrange("p (a s) -> p a s", s=S)
            bA = spool.tile([128, 90 * 16], bf16, tag="treeBA")
            bA3 = bA[:, :].rearrange("p (a c) -> p a c", c=16)
            nc.vector.tensor_tensor(bA3, pbz[:, :, 0:16], pbz[:, :, 16:32],
                                    Alu.add)
            bB = spool.tile([128, 90 * 8], bf16, tag="treeBB")
            bB3 = bB[:, :].rearrange("p (a c) -> p a c", c=8)
            nc.vector.tensor_tensor(bB3, bA3[:, :, 0:8], bA3[:, :, 8:16],
                                    Alu.add)
            bC = spool.tile([128, 90 * 4], bf16, tag="treeBC")
            bC3 = bC[:, :].rearrange("p (a c) -> p a c", c=4)
            nc.vector.tensor_tensor(bC3, bB3[:, :, 0:4], bB3[:, :, 4:8],
                                    Alu.add)
            # tail s=32..35 pairs
            bT = spool.tile([128, 90 * 2], bf16, tag="treeBT")
            bT3 = bT[:, :].rearrange("p (a c) -> p a c", c=2)
            nc.vector.tensor_tensor(bT3, pbz[:, :, 32:34], pbz[:, :, 34:36],
                                    Alu.add)
            bD = spool.tile([128, 90 * 2], bf16, tag="treeBD")
            bD3 = bD[:, :].rearrange("p (a c) -> p a c", c=2)
            nc.vector.tensor_tensor(bD3, bC3[:, :, 0:2], bC3[:, :, 2:4],
                                    Alu.add)
            bE = spool.tile([128, 90 * 2], f32, tag="treeBE")
            bE3 = bE[:, :].rearrange("p (a c) -> p a c", c=2)
            nc.vector.tensor_tensor(bE3, bD3[:, :, :], bT3[:, :, :],
                                    Alu.add)
            t = small.tile([128, J * CP], bf16, tag="tt")
            t3 = t[:, :].rearrange("p (j c) -> p j c", j=J)[:, :, 0:CA]
            nc.vector.tensor_tensor(t3, bE3[:, :, 0], bE3[:, :, 1],
                                    Alu.add)
            return t

        def t_bcast(t):
            """[p, (j, c-padded)] bf16 -> broadcast AP [p, J, D, CA]."""
            return (t[:, :].rearrange("p (j c) -> p j c", j=J)[:, :, 0:CA]
                    .unsqueeze(2).broadcast_to([128, J, D, CA]))

        for ch in range(NCH):
            # ---- iteration 1 (uniform c = 1/J) ----
            xsum = small.tile([128, CA], bf16, tag="xsum")
            with nc.allow_low_precision("bf16 routing intermediates"):
                nc.vector.tensor_reduce(
                    xsum[:, :],
                    Xcs[ch][:, :].rearrange("p (c s) -> p c s", c=CA), AxX,
                    Alu.add)
            xs1 = small.tile([128, CA], bf16, tag="xsum1")
            nc.scalar.mul(xs1[:, :], xsum[:, :], 1.0 / J)
            xs_b = (xs1[:, :].unsqueeze(1).unsqueeze(1)
                    .broadcast_to([128, J, D, CA]))
            s_ps, s_sb = c_step(ch, xs_b)
            vt = squash(ch, s_ps, s_sb, want_bf16=True)
            u = u_step(ch, vt)
            e_heavy(ch, u, L[ch][:, :])  # L = db1  (b was zero)

            # ---- iteration 2 ----
            ct = softmax(ch)
            t = b_heavy(ch, ct)
            s_ps, s_sb = c_step(ch, t_bcast(t))
            vt = squash(ch, s_ps, s_sb, want_bf16=True)
            u = u_step(ch, vt)
            db = spool.tile([128, J * S], f32, tag="db2")
            e_heavy(ch, u, db[:, :])
            nc.vector.tensor_tensor(L[ch][:, :], L[ch][:, :], db[:, :],
                                    Alu.add)

            # ---- iteration 3 (only v needed) ----
            ct = softmax(ch)
            t = b_heavy(ch, ct)
            s_ps, s_sb = c_step(ch, t_bcast(t))
            vt = squash(ch, s_ps, s_sb, want_bf16=False)
            dmae.dma_start(v_d[ch * B4:(ch + 1) * B4, :],
                                vt[0:128:NS, :])

    if split_waits:
        _split_multi_waits(nc)
    return nc


def _split_multi_waits(nc):
    """Walrus's cayman codegen allows exactly ONE sync wait per TPB
    instruction (NEURON_ISA_TPB_EVENTS has a single wait slot). Tile's
    scheduler attaches several waits to dependency-merge instructions,
    which the native bass encoder handles but the neuronx-cc path rejects
    ("Too many sync wait commands"). Split the extras onto engine-local
    NoOp instructions inserted immediately before the owner so the wait
    semantics are unchanged.
    """
    from concourse import mybir

    for bbname, bbwrap in nc.bb_map.items():
        bb = bbwrap.bb
        insts = bb.instructions
        i = 0
        while i < len(insts):
            ins = insts[i]
            si = getattr(ins, "sync_info", None)
            if si is None or len(si.on_wait or []) <= 1:
                i += 1
                continue
            waits = list(si.on_wait)
            engine = ins.engine
            for w in waits[:-1]:
                nop = mybir.InstNoOp(
                    name=nc.get_next_instruction_name(),
                    engine=engine,
                    bass_nofuse=True,
                    sync_info=mybir.SyncInfo(on_wait=[w], on_update=[]),
                )
                insts.insert(i, nop)
                i += 1
            ins.sync_info = mybir.SyncInfo(on_wait=[waits[-1]],
                                           on_update=si.on_update)
            i += 1


def _get_program(split_waits=True, dve_chunks=8, dma_eng="sync"):
    key = ("nc", split_waits, dve_chunks, dma_eng)
    if key not in _CACHE:
        _CACHE[key] = _build_program(split_waits, dve_chunks, dma_eng)
    return _CACHE[key]


def _host_prep(x, W, bias):
    """Build per-core input maps."""
    bf = np.float16
    x = np.ascontiguousarray(x, dtype=np.float32)
    W = np.ascontiguousarray(W, dtype=np.float32)
    bias = np.ascontiguousarray(bias, dtype=np.float32)
    bs = x.shape[0]

    xga = x.reshape(bs, NS, C_IN, S)
    xa = np.concatenate(
        [xga, np.ones((bs, NS, 1, S), dtype=np.float32)], axis=2)
    # [core, ch, b4, g, c, s]
    x6 = xa.reshape(NCORES, NCH, B4, NS, CA, S)
    xcs = np.ascontiguousarray(x6).reshape(
        NCORES, NCH, 128, CA * S).astype(bf)
    x6sc = x6.transpose(0, 1, 2, 3, 5, 4)      # [.., s, c]
    x6sp = np.concatenate(
        [x6sc, np.zeros(x6sc.shape[:-1] + (CP - CA,), np.float32)], axis=-1)
    xsc = np.ascontiguousarray(x6sp).reshape(
        NCORES, NCH, 128, S * CP).astype(bf)

    Wa = np.concatenate(
        [W.reshape(NS, J, D, C_IN),
         bias.reshape(NS, J, D, 1)], axis=3)            # [g, j, d, c]
    Wap = np.concatenate(
        [Wa, np.zeros(Wa.shape[:-1] + (CP - CA,), np.float32)], axis=-1)
    wc = np.tile(Wap.reshape(NS, J * D * CP), (B4, 1)).astype(bf)
    wu = np.tile(
        Wa.transpose(0, 1, 3, 2).reshape(NS, J * CA * D),
        (B4, 1)).astype(bf)                             # [128, (j,c,d)]
    onesb = np.kron(np.eye(B4, dtype=np.float32),
                    np.ones((NS, NS), dtype=np.float32)).astype(bf)

    in_maps = [
        {"xcs": np.ascontiguousarray(xcs[k]),
         "xsc": np.ascontiguousarray(xsc[k]),
         "wc": wc, "wu": wu, "onesb": onesb}
        for k in range(NCORES)
    ]
    return in_maps


def kernel(x, W, bias, b0):
    from concourse.bass_utils import run_bass_kernel_spmd

    nc = _get_program()
    in_maps = _host_prep(x, W, bias)
    res = run_bass_kernel_spmd(nc, in_maps, list(range(NCORES)))
    out = np.concatenate([res.results[k]["v"] for k in range(NCORES)],
                         axis=0)
    return np.ascontiguousarray(out.reshape(NCORES * BLOC, J, D))



# revision 8
# speedup vs baseline: 1.6837x; 1.6837x over previous
# Trainium2 Bass kernel for nn_CapLayer (CapsNet grouped 1x1 conv + dynamic routing).
#
# Key algebraic restructuring: the huge intermediate pred[b, i=(g,s), (j,d)]
# (188MB for the full batch) is NEVER materialized. Routing is computed in a
# factored form:
#   pred[b,(g,s),(j,d)] = sum_c Wa[g,j,d,c] * xga[b,g,c,s]     (c augmented with
#                                                               a ones channel to
#                                                               absorb the bias)
#   t[b,j,g,c]  = sum_s c[b,j,(g,s)] * xga[b,g,c,s]
#   s[b,j,d]    = sum_{g,c} t[b,j,g,c] * Wa[g,j,d,c]
#   u[b,j,g,c]  = sum_d v[b,j,d] * Wa[g,j,d,c]
#   db[b,j,g,s] = sum_c u[b,j,g,c] * xga[b,g,c,s]
# Iteration 1 collapses (softmax of zeros is uniform): t1 = xsum / J.
#
# Sharding: pure data parallel, 32 samples per core across 8 cores.
# On-chip layout: partition p = (b4, g) with 4 samples x 32 groups = 128
# partitions; 8 chunks cover the 32 local samples. The g-contraction for
# s[b,(j,d)] is done on the TensorEngine with a block-diagonal ones matrix,
# which also replicates s across the g-partitions for free (so v and u stay
# in the same partition layout).
#
# Perf structure (v2):
#  - Emission is STEP-INTERLEAVED across the 8 chunks: every engine sees 8
#    independent instances of each step back-to-back, so cross-engine
#    round-trips (DVE<->Act<->PE) never stall the bottleneck engine.
#  - Per-(chunk, iteration) engine routing: a tunable plan assigns whole
#    routing iterations of some chunks to GPSIMD (Pool) to offload the DVE.
#    Each engine class gets its own tile pools so the two never share
#    buffer rings (no cross-engine WAR stalls).
#  - Routing logits, softmax and products run in bf16 (DVE 2x mode);
#    reciprocals ride the Activation engine (Reciprocal activation), which
#    also absorbs exp/sqrt/square/copies.

import sys

import numpy as np

# concourse (Bass/Tile) ships with the container; make sure it's importable
# when the grader runs kernel.py from a bare directory.
for _p in ("/opt/trn_rl_repo", "/root/.axon_site/_ro/trn_rl_repo"):
    if _p not in sys.path:
        sys.path.insert(0, _p)

NS, J, D, C_IN, H, WID, RN = 32, 10, 16, 8, 6, 6, 3
S = H * WID            # 36 spatial positions
CA = C_IN + 1          # 9 channels including the ones channel
CP = 10                # padded channel stride (4B alignment for bf16 rows)
NCORES = 8
BLOC = 32              # samples per core
B4 = 4                 # samples per chunk
NCH = BLOC // B4       # 8 chunks

_CACHE = {}

# Default engine plan: plan[(ch, it)] -> 'v' (DVE) or 'p' (Pool/GPSIMD).
# Chosen to balance DVE ~170us vs Pool ~165us (Pool is ~3x slower per
# element, so it takes ~1/4 of the element work).
def _default_plan():
    plan = {}
    for ch in range(NCH):
        for it in (1, 2, 3):
            plan[(ch, it)] = "v"
    # ~2 chunks per iteration ride the Pool engine, never the same chunk
    # in consecutive iterations (a chunk's iterations are serial).
    for ch in (6, 7):
        plan[(ch, 1)] = "p"
    for ch in (0, 1):
        plan[(ch, 2)] = "p"
    for ch in (3, 4):
        plan[(ch, 3)] = "p"
    return plan


def _order(plan, it):
    # Pool-routed chunks first: their serial chains are ~3x longer, so they
    # must start as early as possible within each phase.
    return sorted(range(NCH), key=lambda ch: plan[(ch, it)] != "p")


def _build_program(split_waits=True, plan=None, dma_eng="sync"):
    from contextlib import ExitStack

    import concourse.bass as bass
    import concourse.tile as tile
    from concourse import mybir

    if plan is None:
        plan = _default_plan()

    f32 = mybir.dt.float32
    bf16 = mybir.dt.float16
    Alu = mybir.AluOpType
    Act = mybir.ActivationFunctionType
    AxX = mybir.AxisListType.X

    nc = bass.Bass("TRN2", target_bir_lowering=True, debug=False,
                   num_devices=NCORES)

    xcs_d = nc.dram_tensor("xcs", [NCH, 128, CA * S], bf16,
                           kind="ExternalInput").ap()      # free = (c, s)
    xsc_d = nc.dram_tensor("xsc", [NCH, 128, S * CP], bf16,
                           kind="ExternalInput").ap()      # free = (s, c10)
    wc_d = nc.dram_tensor("wc", [128, J * D * CP], bf16,
                          kind="ExternalInput").ap()       # free = (j, d, c10)
    wu_d = nc.dram_tensor("wu", [128, J * CA * D], bf16,
                          kind="ExternalInput").ap()       # free = (j, c, d)
    ones_d = nc.dram_tensor("onesb", [128, 128], bf16,
                            kind="ExternalInput").ap()     # blockdiag over b4
    v_d = nc.dram_tensor("v", [BLOC, J * D], f32,
                         kind="ExternalOutput").ap()

    dmae = {"gpsimd": nc.gpsimd, "sync": nc.sync}[dma_eng]
    engs = {"v": nc.vector, "p": nc.gpsimd}

    with tile.TileContext(nc) as tc, ExitStack() as ctx, \
            nc.allow_low_precision("bf16 routing intermediates"):
        consts = ctx.enter_context(tc.tile_pool(name="consts", bufs=1))
        xpool = ctx.enter_context(tc.tile_pool(name="xpool", bufs=1))
        lpool = ctx.enter_context(tc.tile_pool(name="lpool", bufs=1))
        # per-engine-class scratch pools (so DVE and Pool never share rings)
        sv = ctx.enter_context(tc.tile_pool(name="sv", bufs=2))
        sp = ctx.enter_context(tc.tile_pool(name="sp", bufs=2))
        smv = ctx.enter_context(tc.tile_pool(name="smv", bufs=4))
        smp = ctx.enter_context(tc.tile_pool(name="smp", bufs=4))
        vpv = ctx.enter_context(tc.tile_pool(name="vpv", bufs=3))
        vpp = ctx.enter_context(tc.tile_pool(name="vpp", bufs=3))
        psum = ctx.enter_context(tc.tile_pool(name="psum", bufs=4,
                                              space="PSUM"))

        SCR = {"v": sv, "p": sp}
        SML = {"v": smv, "p": smp}
        VPO = {"v": vpv, "p": vpp}

        wc_t = consts.tile([128, J * D * CP], bf16, tag="wc")
        dmae.dma_start(wc_t[:, :], wc_d[:, :])
        wu_t = consts.tile([128, J * CA * D], bf16, tag="wu")
        dmae.dma_start(wu_t[:, :], wu_d[:, :])
        ones_t = consts.tile([128, 128], bf16, tag="onesb")
        dmae.dma_start(ones_t[:, :], ones_d[:, :])

        # Persistent per-chunk tiles.
        Xcs = []   # xga [p, (c, s)] bf16
        Xsc = []   # xga [p, (s, c)] bf16
        L = []     # routing logits b, layout [p, (j, s)] bf16
        for ch in range(NCH):
            xt = xpool.tile([128, CA * S], bf16, tag=f"Xcs{ch}",
                            name=f"Xcs{ch}")
            dmae.dma_start(xt[:, :], xcs_d[ch, :, :])
            Xcs.append(xt)
            xt2 = xpool.tile([128, S * CP], bf16, tag=f"Xsc{ch}",
                             name=f"Xsc{ch}")
            dmae.dma_start(xt2[:, :], xsc_d[ch, :, :])
            Xsc.append(xt2)
            L.append(lpool.tile([128, J * S], bf16, tag=f"L{ch}",
                                name=f"L{ch}"))

        def E(ch, it):
            return engs[plan[(ch, it)]]

        def c_step(ch, it, t_in0_bcast):
            """t x Wa summed over (g, c) -> replicated s [p, (j,d)].

            t_in0_bcast: AP broadcast to [p, J, D, CA] (bf16).
            Returns (psum_tile, sbuf bf16 copy) with s replicated over g
            within each b4 partition block.
            """
            k = plan[(ch, it)]
            eng = engs[k]
            pc = SCR[k].tile([128, J * D * CP], bf16, tag="prodC")
            pc4 = (pc[:, :].rearrange("p (j d c) -> p j d c", j=J, d=D)
                   [:, :, :, 0:CA])
            wc4 = (wc_t[:, :].rearrange("p (j d c) -> p j d c", j=J, d=D)
                   [:, :, :, 0:CA])
            eng.tensor_tensor(pc4, t_in0_bcast, wc4, Alu.mult)
            # PE contracts g (partitions, via blockdiag ones) AND c (PSUM
            # accumulation over the 9 channel slices) in one group.
            pcz = pc[:, :].rearrange("p (a c) -> p a c", c=CP)
            ps = psum.tile([128, J * D], f32, tag="psum_s")
            for c in range(CA):
                nc.tensor.matmul(ps[:, :], ones_t[:, :], pcz[:, :, c],
                                 start=(c == 0), stop=(c == CA - 1))
            s_sb = SML[k].tile([128, J * D], bf16, tag="s_sb")
            nc.scalar.copy(s_sb[:, :], ps[:, :])
            return ps, s_sb

        def squash(ch, it, s_ps, s_sb, want_bf16):
            """v = s * |s| / (1 + |s|^2), norm over d."""
            k = plan[(ch, it)]
            eng = engs[k]
            s2 = SML[k].tile([128, J * D], f32, tag="s2")
            nc.scalar.activation(s2[:, :], s_ps[:, :], Act.Square)
            n2 = SML[k].tile([128, J], f32, tag="n2")
            nc.vector.tensor_reduce(
                n2[:, :], s2[:, :].rearrange("p (j d) -> p j d", j=J), AxX,
                Alu.add)
            n2p1 = SML[k].tile([128, J], f32, tag="n2p1")
            nc.scalar.add(n2p1[:, :], n2[:, :], 1.0)
            r = SML[k].tile([128, J], f32, tag="rcp")
            nc.vector.reciprocal(r[:, :], n2p1[:, :])
            nr = SML[k].tile([128, J], f32, tag="nrm")
            nc.scalar.activation(nr[:, :], n2[:, :], Act.Sqrt)
            f = SML[k].tile([128, J], f32, tag="fac")
            eng.tensor_tensor(f[:, :], nr[:, :], r[:, :], Alu.mult)
            fb = f[:, :].unsqueeze(2).broadcast_to([128, J, D])
            if want_bf16:
                vt = VPO[k].tile([128, J * D], bf16, tag="vtb")
            else:
                vt = VPO[k].tile([128, J * D], f32, tag="vtf")
            eng.tensor_tensor(
                vt[:, :].rearrange("p (j d) -> p j d", j=J),
                s_sb[:, :].rearrange("p (j d) -> p j d", j=J), fb, Alu.mult)
            return vt

        def u_step(ch, it, vt):
            """u[p,(j,c)] = sum_d v[p,(j,d)] * Wa[p,(j,c,d)]. Out bf16
            padded to stride CP."""
            k = plan[(ch, it)]
            eng = engs[k]
            pu = SCR[k].tile([128, J * CA * D], bf16, tag="produ")
            pu4 = pu[:, :].rearrange("p (j c d) -> p j c d", j=J, c=CA)
            wu4 = wu_t[:, :].rearrange("p (j c d) -> p j c d", j=J, c=CA)
            vb = (vt[:, :].rearrange("p (j d) -> p j d", j=J)
                  .unsqueeze(2).broadcast_to([128, J, CA, D]))
            eng.tensor_tensor(pu4, vb, wu4, Alu.mult)
            puz = pu[:, :].rearrange("p (a d) -> p a d", d=D)
            uA = SCR[k].tile([128, 90 * 8], bf16, tag="treeUA")
            uA3 = uA[:, :].rearrange("p (a c) -> p a c", c=8)
            eng.tensor_tensor(uA3, puz[:, :, 0:8], puz[:, :, 8:16],
                              Alu.add)
            uB = SCR[k].tile([128, 90 * 4], bf16, tag="treeUB")
            uB3 = uB[:, :].rearrange("p (a c) -> p a c", c=4)
            eng.tensor_tensor(uB3, uA3[:, :, 0:4], uA3[:, :, 4:8],
                              Alu.add)
            uC = SCR[k].tile([128, 90 * 2], bf16, tag="treeUC")
            uC3 = uC[:, :].rearrange("p (a c) -> p a c", c=2)
            eng.tensor_tensor(uC3, uB3[:, :, 0:2], uB3[:, :, 2:4],
                              Alu.add)
            u = SML[k].tile([128, J * CP], bf16, tag="u")
            u3 = u[:, :].rearrange("p (j c) -> p j c", j=J)[:, :, 0:CA]
            eng.tensor_tensor(u3, uC3[:, :, 0], uC3[:, :, 1],
                              Alu.add)
            return u

        def e_heavy(ch, it, u, out_js, accum):
            """db[p,(j,s)] = sum_c u[p,(j,c)] * x[p,(s,c)].

            accum=False: out_js = db (fresh write, iter 1 -> L).
            accum=True:  out_js += db (iter 2 updates L in place)."""
            k = plan[(ch, it)]
            eng = engs[k]
            pe = SCR[k].tile([128, J * S * CP], bf16, tag="prodE")
            pe4 = (pe[:, :].rearrange("p (j s c) -> p j s c", j=J, s=S)
                   [:, :, :, 0:CA])
            ub = (u[:, :].rearrange("p (j c) -> p j c", j=J)[:, :, 0:CA]
                  .unsqueeze(2).broadcast_to([128, J, S, CA]))
            xb = (Xsc[ch][:, :].rearrange("p (s c) -> p s c", s=S)
                  [:, :, 0:CA].unsqueeze(1)
                  .broadcast_to([128, J, S, CA]))
            eng.tensor_tensor(pe4, ub, xb, Alu.mult)
            # pe layout (j, s, c10): (j,s) merges; tree-sum over c
            pez = pe[:, :].rearrange("p (a c) -> p a c", c=CP)
            eA = SCR[k].tile([128, 360 * 4], bf16, tag="treeEA")
            eA3 = eA[:, :].rearrange("p (a c) -> p a c", c=4)
            eng.tensor_tensor(eA3, pez[:, :, 0:4], pez[:, :, 4:8],
                              Alu.add)
            eB = SCR[k].tile([128, 360 * 2], bf16, tag="treeEB")
            eB3 = eB[:, :].rearrange("p (a c) -> p a c", c=2)
            eng.tensor_tensor(eB3, eA3[:, :, 0:2], eA3[:, :, 2:4],
                              Alu.add)
            if accum:
                db = SCR[k].tile([128, J * S], bf16, tag="db2")
                eng.tensor_tensor(db[:, :], eB3[:, :, 0], eB3[:, :, 1],
                                  Alu.add)
                if k == "v":
                    eng.scalar_tensor_tensor(db[:, :], pez[:, :, 8], 1.0,
                                             db[:, :], Alu.mult, Alu.add)
                else:
                    # neuronx-cc rejects TensorScalarPtr on Pool
                    eng.tensor_tensor(db[:, :], db[:, :], pez[:, :, 8],
                                      Alu.add)
                eng.tensor_tensor(out_js, out_js, db[:, :], Alu.add)
            else:
                eng.tensor_tensor(out_js, eB3[:, :, 0], eB3[:, :, 1],
                                  Alu.add)
                if k == "v":
                    eng.scalar_tensor_tensor(out_js, pez[:, :, 8], 1.0,
                                             out_js, Alu.mult, Alu.add)
                else:
                    eng.tensor_tensor(out_js, out_js, pez[:, :, 8],
                                      Alu.add)

        def softmax(ch, it):
            """c[p,(j,s)] = softmax_j(L). Returns bf16 C tile."""
            k = plan[(ch, it)]
            eng = engs[k]
            et = SCR[k].tile([128, J * S], bf16, tag="expt")
            nc.scalar.activation(et[:, :], L[ch][:, :], Act.Exp)
            z = SML[k].tile([128, S], f32, tag="z")
            # reduce over j: view [p, s(outer, stride 1), j(inner, stride S)]
            ejs = (et[:, :].rearrange("p (j s) -> p j s", j=J)
                   .transpose([0, 2, 1]))
            nc.vector.tensor_reduce(z[:, :], ejs, AxX, Alu.add)
            zr = SML[k].tile([128, S], bf16, tag="zr")
            nc.vector.reciprocal(zr[:, :], z[:, :])
            ct = SCR[k].tile([128, J * S], bf16, tag="ct")
            zb = zr[:, :].unsqueeze(1).broadcast_to([128, J, S])
            eng.tensor_tensor(
                ct[:, :].rearrange("p (j s) -> p j s", j=J),
                et[:, :].rearrange("p (j s) -> p j s", j=J), zb, Alu.mult)
            return ct

        def b_heavy(ch, it, ct):
            """t[p,(j,c)] = sum_s c[p,(j,s)] * x[p,(c,s)]. Out bf16 padded
            to stride CP."""
            k = plan[(ch, it)]
            eng = engs[k]
            pb = SCR[k].tile([128, J * CA * S], bf16, tag="prodB")
            pb4 = pb[:, :].rearrange("p (j c s) -> p j c s", j=J, c=CA)
            cb = (ct[:, :].rearrange("p (j s) -> p j s", j=J)
                  .unsqueeze(2).broadcast_to([128, J, CA, S]))
            xb = (Xcs[ch][:, :].rearrange("p (c s) -> p c s", c=CA)
                  .unsqueeze(1).broadcast_to([128, J, CA, S]))
            eng.tensor_tensor(pb4, cb, xb, Alu.mult)
            pbz = pb[:, :].rearrange("p (a s) -> p a s", s=S)
            bA = SCR[k].tile([128, 90 * 16], bf16, tag="treeBA")
            bA3 = bA[:, :].rearrange("p (a c) -> p a c", c=16)
            eng.tensor_tensor(bA3, pbz[:, :, 0:16], pbz[:, :, 16:32],
                              Alu.add)
            bB = SCR[k].tile([128, 90 * 8], bf16, tag="treeBB")
            bB3 = bB[:, :].rearrange("p (a c) -> p a c", c=8)
            eng.tensor_tensor(bB3, bA3[:, :, 0:8], bA3[:, :, 8:16],
                              Alu.add)
            bC = SCR[k].tile([128, 90 * 4], bf16, tag="treeBC")
            bC3 = bC[:, :].rearrange("p (a c) -> p a c", c=4)
            eng.tensor_tensor(bC3, bB3[:, :, 0:4], bB3[:, :, 4:8],
                              Alu.add)
            # tail s=32..35 pairs
            bT = SCR[k].tile([128, 90 * 2], bf16, tag="treeBT")
            bT3 = bT[:, :].rearrange("p (a c) -> p a c", c=2)
            eng.tensor_tensor(bT3, pbz[:, :, 32:34], pbz[:, :, 34:36],
                              Alu.add)
            bD = SCR[k].tile([128, 90 * 2], bf16, tag="treeBD")
            bD3 = bD[:, :].rearrange("p (a c) -> p a c", c=2)
            eng.tensor_tensor(bD3, bC3[:, :, 0:2], bC3[:, :, 2:4],
                              Alu.add)
            bE = SCR[k].tile([128, 90 * 2], bf16, tag="treeBE")
            bE3 = bE[:, :].rearrange("p (a c) -> p a c", c=2)
            eng.tensor_tensor(bE3, bD3[:, :, :], bT3[:, :, :],
                              Alu.add)
            t = SML[k].tile([128, J * CP], bf16, tag="tt")
            t3 = t[:, :].rearrange("p (j c) -> p j c", j=J)[:, :, 0:CA]
            eng.tensor_tensor(t3, bE3[:, :, 0], bE3[:, :, 1],
                              Alu.add)
            return t

        def t_bcast(t):
            """[p, (j, c-padded)] bf16 -> broadcast AP [p, J, D, CA]."""
            return (t[:, :].rearrange("p (j c) -> p j c", j=J)[:, :, 0:CA]
                    .unsqueeze(2).broadcast_to([128, J, D, CA]))

        # ---------------- step-interleaved emission ----------------
        # per-chunk state carried between phases
        xs_b = [None] * NCH
        s_cur = [None] * NCH
        vt_cur = [None] * NCH
        u_cur = [None] * NCH
        ct_cur = [None] * NCH
        t_cur = [None] * NCH

        # ---- iteration 1 (uniform c = 1/J) ----
        ord1 = _order(plan, 1)
        ord2 = _order(plan, 2)
        ord3 = _order(plan, 3)
        for ch in ord1:
            k = plan[(ch, 1)]
            xsum = SML[k].tile([128, CA], bf16, tag="xsum")
            nc.vector.tensor_reduce(
                xsum[:, :],
                Xcs[ch][:, :].rearrange("p (c s) -> p c s", c=CA), AxX,
                Alu.add)
            xs1 = SML[k].tile([128, CA], bf16, tag="xsum1")
            nc.scalar.mul(xs1[:, :], xsum[:, :], 1.0 / J)
            xs_b[ch] = (xs1[:, :].unsqueeze(1).unsqueeze(1)
                        .broadcast_to([128, J, D, CA]))
        for ch in ord1:
            s_cur[ch] = c_step(ch, 1, xs_b[ch])
        for ch in ord1:
            vt_cur[ch] = squash(ch, 1, s_cur[ch][0], s_cur[ch][1],
                                want_bf16=True)
        for ch in ord1:
            u_cur[ch] = u_step(ch, 1, vt_cur[ch])
        for ch in ord1:
            e_heavy(ch, 1, u_cur[ch], L[ch][:, :], accum=False)

        # ---- iteration 2 ----
        for ch in ord2:
            ct_cur[ch] = softmax(ch, 2)
        for ch in ord2:
            t_cur[ch] = b_heavy(ch, 2, ct_cur[ch])
        for ch in ord2:
            s_cur[ch] = c_step(ch, 2, t_bcast(t_cur[ch]))
        for ch in ord2:
            vt_cur[ch] = squash(ch, 2, s_cur[ch][0], s_cur[ch][1],
                                want_bf16=True)
        for ch in ord2:
            u_cur[ch] = u_step(ch, 2, vt_cur[ch])
        for ch in ord2:
            e_heavy(ch, 2, u_cur[ch], L[ch][:, :], accum=True)

        # ---- iteration 3 (only v needed) ----
        for ch in ord3:
            ct_cur[ch] = softmax(ch, 3)
        for ch in ord3:
            t_cur[ch] = b_heavy(ch, 3, ct_cur[ch])
        for ch in ord3:
            s_cur[ch] = c_step(ch, 3, t_bcast(t_cur[ch]))
        for ch in ord3:
            vt_cur[ch] = squash(ch, 3, s_cur[ch][0], s_cur[ch][1],
                                want_bf16=False)
            dmae.dma_start(v_d[ch * B4:(ch + 1) * B4, :],
                           vt_cur[ch][0:128:NS, :])

    if split_waits:
        _split_multi_waits(nc)
    return nc


def _split_multi_waits(nc):
    """Walrus's cayman codegen allows exactly ONE sync wait per TPB
    instruction (NEURON_ISA_TPB_EVENTS has a single wait slot). Tile's
    scheduler attaches several waits to dependency-merge instructions,
    which the native bass encoder handles but the neuronx-cc path rejects
    ("Too many sync wait commands"). Split the extras onto engine-local
    NoOp instructions inserted immediately before the owner so the wait
    semantics are unchanged.
    """
    from concourse import mybir

    for bbname, bbwrap in nc.bb_map.items():
        bb = bbwrap.bb
        insts = bb.instructions
        i = 0
        while i < len(insts):
            ins = insts[i]
            si = getattr(ins, "sync_info", None)
            if si is None or len(si.on_wait or []) <= 1:
                i += 1
                continue
            waits = list(si.on_wait)
            engine = ins.engine
            for w in waits[:-1]:
                nop = mybir.InstNoOp(
                    name=nc.get_next_instruction_name(),
                    engine=engine,
                    bass_nofuse=True,
                    sync_info=mybir.SyncInfo(on_wait=[w], on_update=[]),
                )
                insts.insert(i, nop)
                i += 1
            ins.sync_info = mybir.SyncInfo(on_wait=[waits[-1]],
                                           on_update=si.on_update)
            i += 1


def _get_program(split_waits=True, plan=None, dma_eng="sync"):
    key = ("nc", split_waits, dma_eng)
    if key not in _CACHE:
        _CACHE[key] = _build_program(split_waits, plan, dma_eng)
    return _CACHE[key]


def _host_prep(x, W, bias):
    """Build per-core input maps."""
    bf = np.float16
    x = np.ascontiguousarray(x, dtype=np.float32)
    W = np.ascontiguousarray(W, dtype=np.float32)
    bias = np.ascontiguousarray(bias, dtype=np.float32)
    bs = x.shape[0]

    xga = x.reshape(bs, NS, C_IN, S)
    xa = np.concatenate(
        [xga, np.ones((bs, NS, 1, S), dtype=np.float32)], axis=2)
    # [core, ch, b4, g, c, s]
    x6 = xa.reshape(NCORES, NCH, B4, NS, CA, S)
    xcs = np.ascontiguousarray(x6).reshape(
        NCORES, NCH, 128, CA * S).astype(bf)
    x6sc = x6.transpose(0, 1, 2, 3, 5, 4)      # [.., s, c]
    x6sp = np.concatenate(
        [x6sc, np.zeros(x6sc.shape[:-1] + (CP - CA,), np.float32)], axis=-1)
    xsc = np.ascontiguousarray(x6sp).reshape(
        NCORES, NCH, 128, S * CP).astype(bf)

    Wa = np.concatenate(
        [W.reshape(NS, J, D, C_IN),
         bias.reshape(NS, J, D, 1)], axis=3)            # [g, j, d, c]
    Wap = np.concatenate(
        [Wa, np.zeros(Wa.shape[:-1] + (CP - CA,), np.float32)], axis=-1)
    wc = np.tile(Wap.reshape(NS, J * D * CP), (B4, 1)).astype(bf)
    wu = np.tile(
        Wa.transpose(0, 1, 3, 2).reshape(NS, J * CA * D),
        (B4, 1)).astype(bf)                             # [128, (j,c,d)]
    onesb = np.kron(np.eye(B4, dtype=np.float32),
                    np.ones((NS, NS), dtype=np.float32)).astype(bf)

    in_maps = [
        {"xcs": np.ascontiguousarray(xcs[k]),
         "xsc": np.ascontiguousarray(xsc[k]),
         "wc": wc, "wu": wu, "onesb": onesb}
        for k in range(NCORES)
    ]
    return in_maps


def kernel(x, W, bias, b0):
    from concourse.bass_utils import run_bass_kernel_spmd

    nc = _get_program()
    in_maps = _host_prep(x, W, bias)
    res = run_bass_kernel_spmd(nc, in_maps, list(range(NCORES)))
    out = np.concatenate([res.results[k]["v"] for k in range(NCORES)],
                         axis=0)
    return np.ascontiguousarray(out.reshape(NCORES * BLOC, J, D))


# revision 13
# speedup vs baseline: 1.8404x; 1.0930x over previous
# Trainium2 Bass kernel for nn_CapLayer (CapsNet grouped 1x1 conv + dynamic routing).
#
# Key algebraic restructuring: the huge intermediate pred[b, i=(g,s), (j,d)]
# (188MB for the full batch) is NEVER materialized. Routing is computed in a
# factored form:
#   pred[b,(g,s),(j,d)] = sum_c Wa[g,j,d,c] * xga[b,g,c,s]     (c augmented with
#                                                               a ones channel to
#                                                               absorb the bias)
#   t[b,j,g,c]  = sum_s c[b,j,(g,s)] * xga[b,g,c,s]
#   s[b,j,d]    = sum_{g,c} t[b,j,g,c] * Wa[g,j,d,c]
#   u[b,j,g,c]  = sum_d v[b,j,d] * Wa[g,j,d,c]
#   db[b,j,g,s] = sum_c u[b,j,g,c] * xga[b,g,c,s]
# Iteration 1 collapses (softmax of zeros is uniform): t1 = xsum / J.
#
# Sharding: pure data parallel, 32 samples per core across 8 cores.
# On-chip layout: partition p = (b4, g) with 4 samples x 32 groups = 128
# partitions; 8 chunks cover the 32 local samples.
#
# Perf structure (v4):
#  - Emission is STEP-INTERLEAVED across the 8 chunks; per-(chunk,iteration)
#    engine routing offloads ~1/4 of the element work to GPSIMD (Pool), with
#    separate tile pools per engine class (no shared buffer rings).
#  - s is reduced over g on the TensorEngine twice: once with a blockdiag
#    ones matrix into the replicated [p,(j,d)] layout (iter 3 only, for the
#    output path) and once with a one-hot b4 matrix into the TRANSPOSED
#    layout sT[(j,d), b] for ALL 32 samples at once (iters 1-2).
#  - squash runs in the transposed space: ~6 tiny ops for all 32 samples
#    (instead of per-chunk), giving vT[(j,d), b].
#  - u = v*Wa rides the TensorEngine: per j, matmul(lhsT=vT[d-slice,b],
#    rhs=WaT[d-slice,(g,c)]) -> u0[b,(g,j,c)] in PSUM; Activation downcasts
#    to bf16, and a DRAM bounce scatters u back to the [p=(b4,g), (j,c)]
#    layout (SBUF->SBUF partition scatter is not expressible in one DMA).
#  - Routing logits, softmax and products run in bf16 (DVE 2x mode).

import sys

import numpy as np

# concourse (Bass/Tile) ships with the container; make sure it's importable
# when the grader runs kernel.py from a bare directory.
for _p in ("/opt/trn_rl_repo", "/root/.axon_site/_ro/trn_rl_repo"):
    if _p not in sys.path:
        sys.path.insert(0, _p)

NS, J, D, C_IN, H, WID, RN = 32, 10, 16, 8, 6, 6, 3
S = H * WID            # 36 spatial positions
CA = C_IN + 1          # 9 channels including the ones channel
CP = 10                # padded channel stride (4B alignment for bf16 rows)
NCORES = 8
BLOC = 32              # samples per core
B4 = 4                 # samples per chunk
NCH = BLOC // B4       # 8 chunks
JH = J // 2            # 5 j's per sT half-tile

_CACHE = {}


# Engine plan: plan[(ch, it)] -> 'v' (DVE) or 'p' (Pool/GPSIMD).
def _default_plan():
    plan = {}
    for ch in range(NCH):
        for it in (1, 2, 3):
            plan[(ch, it)] = "v"
    for ch in (6, 7):
        plan[(ch, 1)] = "p"
    for ch in (0, 1):
        plan[(ch, 2)] = "p"
    for ch in (3, 4):
        plan[(ch, 3)] = "p"
    return plan


def _order(plan, it):
    # Pool-routed chunks first: their serial chains are ~3x longer, so they
    # must start as early as possible within each phase.
    return sorted(range(NCH), key=lambda ch: plan[(ch, it)] != "p")


def _build_program(split_waits=True, plan=None, dma_eng="sync"):
    from contextlib import ExitStack

    import concourse.bass as bass
    import concourse.tile as tile
    from concourse import mybir

    if plan is None:
        plan = _default_plan()

    f32 = mybir.dt.float32
    bf16 = mybir.dt.float16
    Alu = mybir.AluOpType
    Act = mybir.ActivationFunctionType
    AxX = mybir.AxisListType.X

    nc = bass.Bass("TRN2", target_bir_lowering=True, debug=False,
                   num_devices=NCORES)

    xcs_d = nc.dram_tensor("xcs", [NCH, 128, CA * S], bf16,
                           kind="ExternalInput").ap()      # free = (c, s)
    xsc_d = nc.dram_tensor("xsc", [NCH, 128, S * CP], bf16,
                           kind="ExternalInput").ap()      # free = (s, c10)
    wc_d = nc.dram_tensor("wc", [128, J * D * CP], bf16,
                          kind="ExternalInput").ap()       # free = (j, d, c10)
    onesb_d = nc.dram_tensor("onesb", [128, 128], bf16,
                             kind="ExternalInput").ap()    # blockdiag over b4
    onest_d = nc.dram_tensor("onest", [128, B4], bf16,
                             kind="ExternalInput").ap()    # one-hot b4
    ones16_d = nc.dram_tensor("ones16", [80, 80], bf16,
                              kind="ExternalInput").ap()   # blockdiag d16
    # block-diagonal over j within a half: wutH[(j',d), (j'',g,c)] =
    # Wa[g, 5H+j'', d, c] * (j' == j'')
    wut0_d = nc.dram_tensor("wut0", [80, NS * JH * CA], bf16,
                            kind="ExternalInput").ap()
    wut1_d = nc.dram_tensor("wut1", [80, NS * JH * CA], bf16,
                            kind="ExternalInput").ap()
    # DRAM bounce buffers for the u scatter (one per routing iteration so
    # WAR between iterations never serializes).
    u0d = [nc.dram_tensor(f"u0d{i}", [BLOC, NS * J * CA], bf16,
                          kind="Internal").ap() for i in (1, 2)]
    v_d = nc.dram_tensor("v", [BLOC, J * D], f32,
                         kind="ExternalOutput").ap()

    dmae = {"gpsimd": nc.gpsimd, "sync": nc.sync}[dma_eng]
    engs = {"v": nc.vector, "p": nc.gpsimd}

    with tile.TileContext(nc) as tc, ExitStack() as ctx, \
            nc.allow_low_precision("bf16 routing intermediates"):
        consts = ctx.enter_context(tc.tile_pool(name="consts", bufs=1))
        xpool = ctx.enter_context(tc.tile_pool(name="xpool", bufs=1))
        lpool = ctx.enter_context(tc.tile_pool(name="lpool", bufs=1))
        sv = ctx.enter_context(tc.tile_pool(name="sv", bufs=2))
        sp = ctx.enter_context(tc.tile_pool(name="sp", bufs=2))
        smv = ctx.enter_context(tc.tile_pool(name="smv", bufs=4))
        smp = ctx.enter_context(tc.tile_pool(name="smp", bufs=4))
        upool = ctx.enter_context(tc.tile_pool(name="upool", bufs=2))
        vpv = ctx.enter_context(tc.tile_pool(name="vpv", bufs=3))
        vpp = ctx.enter_context(tc.tile_pool(name="vpp", bufs=3))
        psum = ctx.enter_context(tc.tile_pool(name="psum", bufs=3,
                                              space="PSUM"))
        pst = ctx.enter_context(tc.tile_pool(name="pst", bufs=1,
                                             space="PSUM"))
        psu = ctx.enter_context(tc.tile_pool(name="psu", bufs=2,
                                             space="PSUM"))

        SCR = {"v": sv, "p": sp}
        SML = {"v": smv, "p": smp}
        VPO = {"v": vpv, "p": vpp}

        wc_t = consts.tile([128, J * D * CP], bf16, tag="wc")
        dmae.dma_start(wc_t[:, :], wc_d[:, :])
        ones_t = consts.tile([128, 128], bf16, tag="onesb")
        dmae.dma_start(ones_t[:, :], onesb_d[:, :])
        onest_t = consts.tile([128, B4], bf16, tag="onest")
        dmae.dma_start(onest_t[:, :], onest_d[:, :])
        ones16_t = consts.tile([80, 80], bf16, tag="ones16")
        dmae.dma_start(ones16_t[:, :], ones16_d[:, :])
        wut_t = []
        for h, dref in ((0, wut0_d), (1, wut1_d)):
            t_ = consts.tile([80, NS * JH * CA], bf16, tag=f"wut{h}")
            dmae.dma_start(t_[:, :], dref[:, :])
            wut_t.append(t_)

        # Persistent x tiles: one big load each, sliced per chunk.
        xcs_all = xpool.tile([128, NCH * CA * S], bf16, tag="xcs_all")
        dmae.dma_start(
            xcs_all[:, :].rearrange("p (ch a) -> p ch a", ch=NCH),
            xcs_d[:, :, :].transpose([1, 0, 2]))
        xsc_all = xpool.tile([128, NCH * S * CP], bf16, tag="xsc_all")
        dmae.dma_start(
            xsc_all[:, :].rearrange("p (ch a) -> p ch a", ch=NCH),
            xsc_d[:, :, :].transpose([1, 0, 2]))
        Xcs = [xcs_all[:, CA * S * ch:CA * S * (ch + 1)]
               for ch in range(NCH)]
        Xsc = [xsc_all[:, S * CP * ch:S * CP * (ch + 1)]
               for ch in range(NCH)]
        L = []     # routing logits b, layout [p, (j, s)] bf16
        for ch in range(NCH):
            L.append(lpool.tile([128, J * S], bf16, tag=f"L{ch}",
                                name=f"L{ch}"))

        def c_prod(ch, it, t_in0_bcast):
            """pc[p,(j,d,c)] = t (broadcast) * Wa."""
            k = plan[(ch, it)]
            eng = engs[k]
            pc = SCR[k].tile([128, J * D * CP], bf16, tag="prodC")
            pc4 = (pc[:, :].rearrange("p (j d c) -> p j d c", j=J, d=D)
                   [:, :, :, 0:CA])
            wc4 = (wc_t[:, :].rearrange("p (j d c) -> p j d c", j=J, d=D)
                   [:, :, :, 0:CA])
            eng.tensor_tensor(pc4, t_in0_bcast, wc4, Alu.mult)
            return pc

        def s_replicated(ch, it, pc):
            """iter-3 path: s summed over (g,c) via blockdiag ones,
            replicated over g -> PSUM [p, (j,d)] + bf16 SBUF copy."""
            k = plan[(ch, it)]
            pcz = pc[:, :].rearrange("p (a c) -> p a c", c=CP)
            ps = psum.tile([128, J * D], f32, tag="psum_s")
            for c in range(CA):
                nc.tensor.matmul(ps[:, :], ones_t[:, :], pcz[:, :, c],
                                 start=(c == 0), stop=(c == CA - 1))
            s_sb = SML[k].tile([128, J * D], bf16, tag="s_sb")
            nc.scalar.copy(s_sb[:, :], ps[:, :])
            return ps, s_sb

        def sT_accum(ch, pos, pc, sT):
            """accumulate this chunk's sT[(j,d), b4-block] into the psum
            tile: sT[jd, 32h + 4*pos + b4] = sum_{g,c} pc[(b4,g), (jd h), c]
            """
            pcz = pc[:, :].rearrange("p (a c) -> p a c", c=CP)
            for h in range(2):
                dst = sT[:, 32 * h + 4 * pos:32 * h + 4 * pos + 4]
                for c in range(CA):
                    nc.tensor.matmul(dst, pcz[:, 80 * h:80 * (h + 1), c],
                                     onest_t[:, :],
                                     start=(c == 0), stop=(c == CA - 1))

        def squash_T(it, sT, hb):
            """Transposed-space squash for one sample half-batch.
            Operates on both j-halves at once via [80, 2x16-col] views.
            Returns a [80, 64] bf16 vT tile (this hb's cols written)."""
            cview = [slice(32 * h + 16 * hb, 32 * h + 16 * (hb + 1))
                     for h in range(2)]
            s2 = smv.tile([80, 64], bf16, tag="s2T")
            n2 = pst.tile([80, 64], f32, tag="n2T")
            for h in range(2):
                nc.scalar.activation(s2[:, cview[h]], sT[:, cview[h]],
                                     Act.Square)
                nc.tensor.matmul(n2[:, cview[h]], ones16_t[:, :],
                                 s2[:, cview[h]], start=True, stop=True)
            n2p1 = smv.tile([80, 64], f32, tag="n2p1T")
            r = smv.tile([80, 64], f32, tag="rT")
            nr = smv.tile([80, 64], f32, tag="nrT")
            f = smv.tile([80, 64], f32, tag="fT")
            vT = smv.tile([80, 64], bf16, tag=f"vT_{it}")
            for h in range(2):
                cv = cview[h]
                nc.scalar.add(n2p1[:, cv], n2[:, cv], 1.0)
                nc.vector.reciprocal(r[:, cv], n2p1[:, cv])
                nc.scalar.activation(nr[:, cv], n2[:, cv], Act.Sqrt)
                nc.vector.tensor_tensor(f[:, cv], nr[:, cv], r[:, cv],
                                        Alu.mult)
                nc.vector.tensor_tensor(vT[:, cv], sT[:, cv], f[:, cv],
                                        Alu.mult)
            return vT

        GCHUNKS = ((0, 10), (10, 20), (20, 30), (30, 32))

        def u_mm_half(it, hb, vT, u0sb):
            """u0[b (16 samples of half hb), (g,j,c)] on the PE via the
            j-blockdiagonal WaT (columns (g, j-in-half, c)), g-chunked for
            the PSUM bank limit, then downcast to bf16 into u0sb
            ([16, NS*J*CA], (g, j, c) layout)."""
            JC = JH * CA          # 45 cols per g per j-half
            for ht in range(2):
                cols = slice(32 * ht + 16 * hb, 32 * ht + 16 * (hb + 1))
                for (g0, g1) in GCHUNKS:
                    ups = psu.tile([16, (g1 - g0) * JC], f32, tag="ups")
                    nc.tensor.matmul(
                        ups[:, :], vT[:, cols],
                        wut_t[ht][:, JC * g0:JC * g1],
                        start=True, stop=True)
                    # scatter into u0sb[(g, j, c)] at j-half offset
                    dst = (u0sb[:, :]
                           .rearrange("p (g j c) -> p g j c", g=NS, j=J)
                           [:, g0:g1, JH * ht:JH * (ht + 1), :])
                    nc.scalar.copy(
                        dst, ups[:, :].rearrange(
                            "p (g j c) -> p g j c", g=g1 - g0, j=JH))

        def u_scatter(it, ch, pos, ush):
            """DRAM-bounce gather: one DMA per chunk. u0d is [32 rows,
            (g,j,c)]; a uniform partition stride of 90 elements walks
            (b4,g) because 32 g-partitions x 90 = 2880 = the row stride."""
            src_ap = (u0d[it - 1][4 * pos:4 * pos + 4, :]
                      .rearrange("b (g a) -> (b g) a", g=NS))
            dmae.dma_start(ush[:, :], src_ap)

        def e_heavy(ch, it, u, out_js, accum):
            """db[p,(j,s)] = sum_c u[p,(j,c)] * x[p,(s,c)].

            accum=False: out_js = db (fresh write, iter 1 -> L).
            accum=True:  out_js += db (iter 2 updates L in place)."""
            k = plan[(ch, it)]
            eng = engs[k]
            pe = SCR[k].tile([128, J * S * CP], bf16, tag="prodE")
            pe4 = (pe[:, :].rearrange("p (j s c) -> p j s c", j=J, s=S)
                   [:, :, :, 0:CA])
            ub = (u[:, :].rearrange("p (j c) -> p j c", c=CA)
                  .unsqueeze(2).broadcast_to([128, J, S, CA]))
            xb = (Xsc[ch].rearrange("p (s c) -> p s c", s=S)
                  [:, :, 0:CA].unsqueeze(1)
                  .broadcast_to([128, J, S, CA]))
            eng.tensor_tensor(pe4, ub, xb, Alu.mult)
            pez = pe[:, :].rearrange("p (a c) -> p a c", c=CP)
            eA = SCR[k].tile([128, 360 * 4], bf16, tag="treeEA")
            eA3 = eA[:, :].rearrange("p (a c) -> p a c", c=4)
            eng.tensor_tensor(eA3, pez[:, :, 0:4], pez[:, :, 4:8],
                              Alu.add)
            eB = SCR[k].tile([128, 360 * 2], bf16, tag="treeEB")
            eB3 = eB[:, :].rearrange("p (a c) -> p a c", c=2)
            eng.tensor_tensor(eB3, eA3[:, :, 0:2], eA3[:, :, 2:4],
                              Alu.add)
            if accum:
                db = SCR[k].tile([128, J * S], bf16, tag="db2")
                eng.tensor_tensor(db[:, :], eB3[:, :, 0], eB3[:, :, 1],
                                  Alu.add)
                if k == "v":
                    eng.scalar_tensor_tensor(db[:, :], pez[:, :, 8], 1.0,
                                             db[:, :], Alu.mult, Alu.add)
                else:
                    eng.tensor_tensor(db[:, :], db[:, :], pez[:, :, 8],
                                      Alu.add)
                eng.tensor_tensor(out_js, out_js, db[:, :], Alu.add)
            else:
                eng.tensor_tensor(out_js, eB3[:, :, 0], eB3[:, :, 1],
                                  Alu.add)
                if k == "v":
                    eng.scalar_tensor_tensor(out_js, pez[:, :, 8], 1.0,
                                             out_js, Alu.mult, Alu.add)
                else:
                    eng.tensor_tensor(out_js, out_js, pez[:, :, 8],
                                      Alu.add)

        def squash_full(ch, it, s_ps, s_sb):
            """Classic squash producing f32 v in [p,(j,d)] (iter 3)."""
            k = plan[(ch, it)]
            eng = engs[k]
            s2 = SML[k].tile([128, J * D], f32, tag="s2")
            nc.scalar.activation(s2[:, :], s_ps[:, :], Act.Square)
            n2 = SML[k].tile([128, J], f32, tag="n2")
            nc.vector.tensor_reduce(
                n2[:, :], s2[:, :].rearrange("p (j d) -> p j d", j=J), AxX,
                Alu.add)
            n2p1 = SML[k].tile([128, J], f32, tag="n2p1")
            nc.scalar.add(n2p1[:, :], n2[:, :], 1.0)
            r = SML[k].tile([128, J], f32, tag="rcp")
            nc.vector.reciprocal(r[:, :], n2p1[:, :])
            nr = SML[k].tile([128, J], f32, tag="nrm")
            nc.scalar.activation(nr[:, :], n2[:, :], Act.Sqrt)
            f = SML[k].tile([128, J], f32, tag="fac")
            eng.tensor_tensor(f[:, :], nr[:, :], r[:, :], Alu.mult)
            fb = f[:, :].unsqueeze(2).broadcast_to([128, J, D])
            vt = VPO[k].tile([128, J * D], f32, tag="vtf")
            eng.tensor_tensor(
                vt[:, :].rearrange("p (j d) -> p j d", j=J),
                s_sb[:, :].rearrange("p (j d) -> p j d", j=J), fb, Alu.mult)
            return vt

        def softmax(ch, it):
            """c[p,(j,s)] = softmax_j(L). Returns bf16 C tile."""
            k = plan[(ch, it)]
            eng = engs[k]
            et = SCR[k].tile([128, J * S], bf16, tag="expt")
            nc.scalar.activation(et[:, :], L[ch][:, :], Act.Exp)
            z = SML[k].tile([128, S], f32, tag="z")
            ejs = (et[:, :].rearrange("p (j s) -> p j s", j=J)
                   .transpose([0, 2, 1]))
            nc.vector.tensor_reduce(z[:, :], ejs, AxX, Alu.add)
            zr = SML[k].tile([128, S], bf16, tag="zr")
            nc.vector.reciprocal(zr[:, :], z[:, :])
            ct = SCR[k].tile([128, J * S], bf16, tag="ct")
            zb = zr[:, :].unsqueeze(1).broadcast_to([128, J, S])
            eng.tensor_tensor(
                ct[:, :].rearrange("p (j s) -> p j s", j=J),
                et[:, :].rearrange("p (j s) -> p j s", j=J), zb, Alu.mult)
            return ct

        def b_heavy(ch, it, ct):
            """t[p,(j,c)] = sum_s c[p,(j,s)] * x[p,(c,s)]."""
            k = plan[(ch, it)]
            eng = engs[k]
            pb = SCR[k].tile([128, J * CA * S], bf16, tag="prodB")
            pb4 = pb[:, :].rearrange("p (j c s) -> p j c s", j=J, c=CA)
            cb = (ct[:, :].rearrange("p (j s) -> p j s", j=J)
                  .unsqueeze(2).broadcast_to([128, J, CA, S]))
            xb = (Xcs[ch].rearrange("p (c s) -> p c s", c=CA)
                  .unsqueeze(1).broadcast_to([128, J, CA, S]))
            eng.tensor_tensor(pb4, cb, xb, Alu.mult)
            pbz = pb[:, :].rearrange("p (a s) -> p a s", s=S)
            bA = SCR[k].tile([128, 90 * 16], bf16, tag="treeBA")
            bA3 = bA[:, :].rearrange("p (a c) -> p a c", c=16)
            eng.tensor_tensor(bA3, pbz[:, :, 0:16], pbz[:, :, 16:32],
                              Alu.add)
            bB = SCR[k].tile([128, 90 * 8], bf16, tag="treeBB")
            bB3 = bB[:, :].rearrange("p (a c) -> p a c", c=8)
            eng.tensor_tensor(bB3, bA3[:, :, 0:8], bA3[:, :, 8:16],
                              Alu.add)
            bC = SCR[k].tile([128, 90 * 4], bf16, tag="treeBC")
            bC3 = bC[:, :].rearrange("p (a c) -> p a c", c=4)
            eng.tensor_tensor(bC3, bB3[:, :, 0:4], bB3[:, :, 4:8],
                              Alu.add)
            bT = SCR[k].tile([128, 90 * 2], bf16, tag="treeBT")
            bT3 = bT[:, :].rearrange("p (a c) -> p a c", c=2)
            eng.tensor_tensor(bT3, pbz[:, :, 32:34], pbz[:, :, 34:36],
                              Alu.add)
            bD = SCR[k].tile([128, 90 * 2], bf16, tag="treeBD")
            bD3 = bD[:, :].rearrange("p (a c) -> p a c", c=2)
            eng.tensor_tensor(bD3, bC3[:, :, 0:2], bC3[:, :, 2:4],
                              Alu.add)
            bE = SCR[k].tile([128, 90 * 2], bf16, tag="treeBE")
            bE3 = bE[:, :].rearrange("p (a c) -> p a c", c=2)
            eng.tensor_tensor(bE3, bD3[:, :, :], bT3[:, :, :],
                              Alu.add)
            t = SML[k].tile([128, J * CP], bf16, tag="tt")
            t3 = t[:, :].rearrange("p (j c) -> p j c", j=J)[:, :, 0:CA]
            eng.tensor_tensor(t3, bE3[:, :, 0], bE3[:, :, 1],
                              Alu.add)
            return t

        def t_bcast(t):
            return (t[:, :].rearrange("p (j c) -> p j c", j=J)[:, :, 0:CA]
                    .unsqueeze(2).broadcast_to([128, J, D, CA]))

        def u_iteration(it, ord_, pos, pcs):
            """iters 1-2: sT accumulate -> squash_T -> u matmuls -> DRAM
            bounce -> per-chunk scatter. Emitted in 2 half-batches of 4
            chunks so the DVE pipeline never drains."""
            sT = pst.tile([80, 64], f32, tag=f"sT_{it}", name=f"sT_{it}")
            ush = {}
            for hb in range(2):
                chs = ord_[4 * hb:4 * hb + 4]
                for ch in chs:
                    sT_accum(ch, pos[ch], pcs[ch], sT)
                vT = squash_T(it, sT, hb)
                u0sb = upool.tile([16, NS * J * CA], bf16, tag=f"u0sb{hb}")
                u_mm_half(it, hb, vT, u0sb)
                dmae.dma_start(
                    u0d[it - 1][16 * hb:16 * (hb + 1), :], u0sb[:, :])
                for ch in chs:
                    k = plan[(ch, it)]
                    u = SCR[k].tile([128, J * CA], bf16, tag="ushuf")
                    u_scatter(it, ch, pos[ch], u)
                    ush[ch] = u
            return ush

        # ---------------- emission ----------------
        ords = {it: _order(plan, it) for it in (1, 2, 3)}
        poss = {it: {ch: i for i, ch in enumerate(ords[it])}
                for it in (1, 2, 3)}
        pcs = [None] * NCH
        ct_cur = [None] * NCH
        t_cur = [None] * NCH

        # ---- iteration 1 (uniform c = 1/J) ----
        ord1 = ords[1]
        for ch in ord1:
            k = plan[(ch, 1)]
            xsum = SML[k].tile([128, CA], bf16, tag="xsum")
            nc.vector.tensor_reduce(
                xsum[:, :],
                Xcs[ch].rearrange("p (c s) -> p c s", c=CA), AxX,
                Alu.add)
            xs1 = SML[k].tile([128, CA], bf16, tag="xsum1")
            nc.scalar.mul(xs1[:, :], xsum[:, :], 1.0 / J)
            xs_b = (xs1[:, :].unsqueeze(1).unsqueeze(1)
                    .broadcast_to([128, J, D, CA]))
            pcs[ch] = c_prod(ch, 1, xs_b)
        ush = u_iteration(1, ord1, poss[1], pcs)
        for ch in ord1:
            e_heavy(ch, 1, ush[ch], L[ch][:, :], accum=False)

        # ---- iteration 2 ----
        ord2 = ords[2]
        for ch in ord2:
            ct_cur[ch] = softmax(ch, 2)
        for ch in ord2:
            t_cur[ch] = b_heavy(ch, 2, ct_cur[ch])
        for ch in ord2:
            pcs[ch] = c_prod(ch, 2, t_bcast(t_cur[ch]))
        ush = u_iteration(2, ord2, poss[2], pcs)
        for ch in ord2:
            e_heavy(ch, 2, ush[ch], L[ch][:, :], accum=True)

        # ---- iteration 3 (only v needed) ----
        ord3 = ords[3]
        for ch in ord3:
            ct_cur[ch] = softmax(ch, 3)
        for ch in ord3:
            t_cur[ch] = b_heavy(ch, 3, ct_cur[ch])
        scur = [None] * NCH
        for ch in ord3:
            pc = c_prod(ch, 3, t_bcast(t_cur[ch]))
            scur[ch] = s_replicated(ch, 3, pc)
        for ch in ord3:
            vt = squash_full(ch, 3, scur[ch][0], scur[ch][1])
            dmae.dma_start(v_d[ch * B4:(ch + 1) * B4, :],
                           vt[0:128:NS, :])

    if split_waits:
        _split_multi_waits(nc)
    return nc


def _split_multi_waits(nc):
    """Walrus's cayman codegen allows exactly ONE sync wait per TPB
    instruction (NEURON_ISA_TPB_EVENTS has a single wait slot). Tile's
    scheduler attaches several waits to dependency-merge instructions,
    which the native bass encoder handles but the neuronx-cc path rejects
    ("Too many sync wait commands"). Split the extras onto engine-local
    NoOp instructions inserted immediately before the owner so the wait
    semantics are unchanged.
    """
    from concourse import mybir

    for bbname, bbwrap in nc.bb_map.items():
        bb = bbwrap.bb
        insts = bb.instructions
        i = 0
        while i < len(insts):
            ins = insts[i]
            si = getattr(ins, "sync_info", None)
            if si is None or len(si.on_wait or []) <= 1:
                i += 1
                continue
            waits = list(si.on_wait)
            engine = ins.engine
            for w in waits[:-1]:
                nop = mybir.InstNoOp(
                    name=nc.get_next_instruction_name(),
                    engine=engine,
                    bass_nofuse=True,
                    sync_info=mybir.SyncInfo(on_wait=[w], on_update=[]),
                )
                insts.insert(i, nop)
                i += 1
            ins.sync_info = mybir.SyncInfo(on_wait=[waits[-1]],
                                           on_update=si.on_update)
            i += 1


def _get_program(split_waits=True, plan=None, dma_eng="sync"):
    key = ("nc", split_waits, dma_eng)
    if key not in _CACHE:
        _CACHE[key] = _build_program(split_waits, plan, dma_eng)
    return _CACHE[key]


def _host_prep(x, W, bias):
    """Build per-core input maps."""
    bf = np.float16
    x = np.ascontiguousarray(x, dtype=np.float32)
    W = np.ascontiguousarray(W, dtype=np.float32)
    bias = np.ascontiguousarray(bias, dtype=np.float32)
    bs = x.shape[0]

    xga = x.reshape(bs, NS, C_IN, S)
    xa = np.concatenate(
        [xga, np.ones((bs, NS, 1, S), dtype=np.float32)], axis=2)
    # [core, ch, b4, g, c, s]
    x6 = xa.reshape(NCORES, NCH, B4, NS, CA, S)
    xcs = np.ascontiguousarray(x6).reshape(
        NCORES, NCH, 128, CA * S).astype(bf)
    x6sc = x6.transpose(0, 1, 2, 3, 5, 4)      # [.., s, c]
    x6sp = np.concatenate(
        [x6sc, np.zeros(x6sc.shape[:-1] + (CP - CA,), np.float32)], axis=-1)
    xsc = np.ascontiguousarray(x6sp).reshape(
        NCORES, NCH, 128, S * CP).astype(bf)

    Wa = np.concatenate(
        [W.reshape(NS, J, D, C_IN),
         bias.reshape(NS, J, D, 1)], axis=3)            # [g, j, d, c]
    Wap = np.concatenate(
        [Wa, np.zeros(Wa.shape[:-1] + (CP - CA,), np.float32)], axis=-1)
    wc = np.tile(Wap.reshape(NS, J * D * CP), (B4, 1)).astype(bf)
    onesb = np.kron(np.eye(B4, dtype=np.float32),
                    np.ones((NS, NS), dtype=np.float32)).astype(bf)
    # one-hot over b4: onest[(b4, g), b4'] = (b4 == b4')
    onest = np.kron(np.eye(B4, dtype=np.float32),
                    np.ones((NS, 1), dtype=np.float32)).astype(bf)
    # blockdiag ones over the 16 d-partitions of each j
    ones16 = np.kron(np.eye(JH, dtype=np.float32),
                     np.ones((D, D), dtype=np.float32)).astype(bf)
    # wutH[(j',d), (g, j'', c)] = Wa[g, 5H+j'', d, c] * (j' == j'')
    wut = np.zeros((2, JH, D, NS, JH, CA), np.float32)
    for h in range(2):
        for jj in range(JH):
            wut[h, jj, :, :, jj, :] = Wa[:, h * JH + jj].transpose(
                1, 0, 2)  # [d, g, c]
    wut0 = np.ascontiguousarray(
        wut[0].reshape(JH * D, NS * JH * CA)).astype(bf)
    wut1 = np.ascontiguousarray(
        wut[1].reshape(JH * D, NS * JH * CA)).astype(bf)

    in_maps = [
        {"xcs": np.ascontiguousarray(xcs[k]),
         "xsc": np.ascontiguousarray(xsc[k]),
         "wc": wc, "onesb": onesb, "onest": onest, "ones16": ones16,
         "wut0": wut0, "wut1": wut1}
        for k in range(NCORES)
    ]
    return in_maps


def kernel(x, W, bias, b0):
    from concourse.bass_utils import run_bass_kernel_spmd

    nc = _get_program()
    in_maps = _host_prep(x, W, bias)
    res = run_bass_kernel_spmd(nc, in_maps, list(range(NCORES)))
    out = np.concatenate([res.results[k]["v"] for k in range(NCORES)],
                         axis=0)
    return np.ascontiguousarray(out.reshape(NCORES * BLOC, J, D))
